# revision 2
# baseline (speedup 1.0000x reference)
"""TRN2 Bass kernel V2 for nn_GCNEModel — pair-packed over shared HBM.

Cores (2k, 2k+1) share an HBM domain on TRN2 (LNC1). Each pair processes its
two samples TOGETHER: the message matrix m2 has 512-byte rows
[sample_even 64 f32 | sample_odd 64 f32], which lifts the edge-gather DMA out
of the sub-512B descriptor penalty (2x less gather time per sample). Each
core owns half the node blocks (balanced degree interleave): it runs GEMM /
aggregation / head for BOTH samples on ITS blocks, scatters its m2 rows into
an addr_space='Shared' HBM tensor, and a pair collective barrier per layer
orders peer scatters before gathers. The k-round gather structure is padded
to a common (role-independent) shape so one SPMD program serves both roles.
Host combines the two partial head outputs per pair.
"""
import os
import sys

os.environ.setdefault("NEURON_RT_RESET_CORES", "1")
for _p in ("/opt/trn_rl_repo", "/root/.axon_site/_ro/trn_rl_repo"):
    if os.path.isdir(_p) and _p not in sys.path:
        sys.path.insert(0, _p)

import numpy as np

import concourse.bacc as bacc
import concourse.mybir as mybir
import concourse.tile as tile
from concourse.bass_utils import run_bass_kernel_spmd
from concourse.tile_rust import add_dep_helper

P = 128
HID = 64
FIN = 36
F2 = 2 * HID          # pair-packed feature width (128)
NFC = 256
N_CORES = 8
NSLOT2 = 120          # 119 real blocks + 1 pad block
NPAD2 = NSLOT2 * P
NLOC = NSLOT2 // 2    # 60 local blocks per core
CHUNK = 7936
PAIRS = [[0, 1], [2, 3], [4, 5], [6, 7]]
PIPELINE = int(os.environ.get("V2_PIPELINE", "1"))
NO_GEMM = 0
NO_TAIL = 0
TAIL_LEVEL = 3


def preprocess(n, edge_index):
    src_old = np.asarray(edge_index[0], np.int64)
    dst_old = np.asarray(edge_index[1], np.int64)
    E = src_old.shape[0]

    deg = np.bincount(dst_old, minlength=n)
    pi = np.argsort(-deg, kind="stable")
    inv_pi = np.empty(n, np.int64)
    inv_pi[pi] = np.arange(n)
    deg_s = np.zeros(NPAD2, np.int64)
    deg_s[:n] = deg[pi]
    dinv = np.zeros(NPAD2)
    dinv[:n] = 1.0 / np.sqrt(deg_s[:n].astype(np.float64) + 1.0)

    src = inv_pi[src_old]
    dst = inv_pi[dst_old]

    blocks = np.arange(NSLOT2)
    core_of_block = np.where((blocks // 2) % 2 == 0,
                             blocks % 2, 1 - (blocks % 2))
    my_blocks = [blocks[core_of_block == r] for r in (0, 1)]

    order = np.argsort(dst, kind="stable")
    src_sorted, dst_sorted = src[order], dst[order]
    starts = np.zeros(n + 1, np.int64)
    np.cumsum(np.bincount(dst, minlength=n), out=starts[1:])
    kpos = np.arange(E) - starts[dst_sorted]

    DUMMY = NPAD2 - 1

    # per-role per-round real-node counts
    role_nk = []
    role_edges = []
    for r in (0, 1):
        mb = my_blocks[r]
        in_half = np.zeros(NSLOT2, bool)
        in_half[mb] = True
        loc_of_block = -np.ones(NSLOT2, np.int64)
        loc_of_block[mb] = np.arange(len(mb))
        sel_e = in_half[dst_sorted // P]
        src_r, dst_r, k_r = src_sorted[sel_e], dst_sorted[sel_e], kpos[sel_e]
        locpos = loc_of_block[dst_r // P] * P + dst_r % P
        mydeg = deg_s[(mb[:, None] * P + np.arange(P)[None, :]).reshape(-1)]
        Kmax = int(mydeg.max()) if len(mydeg) else 0
        nks = [int((mydeg > k).sum()) for k in range(Kmax)]
        role_nk.append(nks)
        role_edges.append((src_r, locpos, k_r))

    # common round structure: nblk_k = max over roles
    Kmax = max(len(role_nk[0]), len(role_nk[1]))
    nblk_common = []
    for k in range(Kmax):
        nb = 0
        for r in (0, 1):
            if k < len(role_nk[r]):
                nb = max(nb, (role_nk[r][k] + P - 1) // P)
        nblk_common.append(nb)

    segments = []          # (stream_blk, nblk) per round, common
    stream_blk = 0
    for nb in nblk_common:
        segments.append((stream_blk, nb))
        stream_blk += nb
    Eprime = stream_blk * P

    chunks = []
    pos = 0
    while pos < Eprime:
        c = min(CHUNK, Eprime - pos)
        chunks.append((pos, c))
        pos += c
    chunk_adds = []
    for (cstart, clen) in chunks:
        c_b0, c_b1 = cstart // P, (cstart + clen) // P
        adds = []
        for (sb, nb) in segments:
            lo, hi = max(c_b0, sb), min(c_b1, sb + nb)
            if lo < hi:
                adds.append((lo - c_b0, lo - sb, hi - lo))
        chunk_adds.append(adds)

    roles = []
    for r in (0, 1):
        src_r, locpos, k_r = role_edges[r]
        nks = role_nk[r]
        idx_stream = np.full(Eprime, DUMMY, np.int64)
        off = 0
        for k, nb in enumerate(nblk_common):
            if k < len(nks):
                n_k = nks[k]
                sel = k_r == k
                srcs_k = src_r[sel][np.argsort(locpos[sel], kind="stable")]
                assert srcs_k.shape[0] == n_k
                lp = np.sort(locpos[sel])
                assert (lp == np.arange(n_k)).all(), f"round {k} not a prefix"
                idx_stream[off:off + n_k] = srcs_k
            off += nb * P
        assert off == Eprime
        cols = Eprime // 16
        arr16 = idx_stream.reshape(cols, 16).T.astype(np.int16)
        gidx_w = np.ascontiguousarray(np.tile(arr16, (8, 1)))
        mb = my_blocks[r]
        sidx = (mb[:, None] * P + np.arange(P)[None, :]).reshape(-1)
        scols = sidx.shape[0] // 16
        sidx_w = np.ascontiguousarray(
            np.tile(sidx.reshape(scols, 16).T.astype(np.int16), (8, 1)))
        roles.append(dict(my_blocks=mb, gidx_w=gidx_w, sidx_w=sidx_w))

    return dict(pi=pi, dinv=dinv, roles=roles, n=n, E=E, Eprime=Eprime,
                chunks=chunks, chunk_adds=chunk_adds, segments=segments)


def build_constants(prep, inputs):
    n = prep["n"]
    pi, dinv = prep["pi"], prep["dinv"]

    pel_W = np.asarray(inputs["pel_W"], np.float32)
    pel_b = np.asarray(inputs["pel_b"], np.float32)
    pe_perm = (pel_W.T + pel_b)[pi]

    x = np.asarray(inputs["x"], np.float32)

    import ml_dtypes
    bf16 = ml_dtypes.bfloat16
    Wc = [np.ascontiguousarray(np.asarray(inputs[f"conv{i}_W"],
                                          np.float32).T.astype(bf16))
          for i in (1, 2, 3)]
    bc2 = [np.ascontiguousarray(np.asarray(inputs[f"conv{i}_b"], np.float32)
                                .reshape(HID, 1)) for i in (1, 2, 3)]
    fc_W = np.asarray(inputs["fc_W"], np.float32).reshape(-1)
    wl2 = [np.ascontiguousarray(fc_W[l::3].reshape(HID, 1).astype(bf16))
           for l in range(3)]

    lin1_W = np.asarray(inputs["lin1_W"], np.float32)
    W1T_full = np.zeros((NPAD2, NFC), bf16)
    W1T_full[:n] = lin1_W[:, pi].T.astype(bf16)

    per_core = []
    for c in range(N_CORES):
        r = c % 2
        role = prep["roles"][r]
        mb = role["my_blocks"]
        rows = (mb[:, None] * P + np.arange(P)[None, :]).reshape(-1)
        s0, s1 = (c // 2) * 2, (c // 2) * 2 + 1
        x2 = np.zeros((FIN, 2, NLOC * P), bf16)
        for si, s in enumerate((s0, s1)):
            xc = np.zeros((NPAD2, FIN), np.float32)
            xc[:n, :4] = x[s][pi]
            xc[:n, 4:] = pe_perm
            x2[:, si, :] = xc[rows].T.astype(bf16)
        dinv_rows = dinv[rows].astype(np.float32)
        dinv64 = np.ascontiguousarray(
            np.repeat(dinv_rows.reshape(NLOC, P).T[:, :, None], HID, axis=2)
        ).reshape(P, NLOC * HID)
        maskv = (rows < n).astype(np.float32)
        mask2 = np.ascontiguousarray(
            np.repeat(maskv.reshape(NLOC, P).T[:, :, None], 2, axis=2)
        ).reshape(P, NLOC * 2)
        W1T_c = np.ascontiguousarray(W1T_full[rows])
        per_core.append(dict(x2=x2, dinv64=dinv64, mask2=mask2,
                             W1T=W1T_c, gidx=role["gidx_w"],
                             sidx=role["sidx_w"]))
    shared = dict(Wc=Wc, bc2=bc2, wl2=wl2)
    return shared, per_core


def build_program(prep, gb_bufs=None, idx_bufs=None, xw_bufs=None,
                  w1_bufs=None, pt_bufs=None, ptr_bufs=None,
                  gemm_grp=4, tr_grp=2, hgrp=4, tile_cores=2):
    _env = os.environ.get
    if gb_bufs is None: gb_bufs = int(_env("V2_GB", "3"))
    if idx_bufs is None: idx_bufs = int(_env("V2_IDXB", "4"))
    if xw_bufs is None: xw_bufs = int(_env("V2_XWB", "2"))
    if w1_bufs is None: w1_bufs = int(_env("V2_W1B", "3"))
    if pt_bufs is None: pt_bufs = int(_env("V2_PTB", "3"))
    if ptr_bufs is None: ptr_bufs = int(_env("V2_PTRB", "3"))
    f32 = mybir.dt.float32
    Eprime = prep["Eprime"]
    chunks = prep["chunks"]
    chunk_adds = prep["chunk_adds"]

    nc = bacc.Bacc("TRN2", debug=False)
    bf = mybir.dt.bfloat16

    x2_d = nc.dram_tensor("x2", [FIN, 2 * NLOC * P], bf, kind="ExternalInput")
    dinv_d = nc.dram_tensor("dinv64", [P, NLOC * HID], f32, kind="ExternalInput")
    mask_d = nc.dram_tensor("mask2", [P, NLOC * 2], f32, kind="ExternalInput")
    Wc_d = [nc.dram_tensor(f"Wc{i}", [FIN if i == 0 else HID, HID], bf,
                           kind="ExternalInput") for i in range(3)]
    bc_d = [nc.dram_tensor(f"bc{i}", [HID, 1], f32, kind="ExternalInput")
            for i in range(3)]
    wl_d = [nc.dram_tensor(f"wl{i}", [HID, 1], bf, kind="ExternalInput")
            for i in range(3)]
    gidx_d = nc.dram_tensor("gidx", [P, Eprime // 16], mybir.dt.int16,
                            kind="ExternalInput")
    sidx_d = nc.dram_tensor("sidx", [P, NLOC * P // 16], mybir.dt.int16,
                            kind="ExternalInput")
    w1t_d = nc.dram_tensor("W1T", [NLOC * P, NFC], bf, kind="ExternalInput")
    ident_d = nc.dram_tensor("ident", [P, P], f32, kind="ExternalInput")
    z_d = nc.dram_tensor("z", [2, NFC], f32, kind="ExternalOutput")

    sh = [nc.dram_tensor(f"sh{l}", [NPAD2, F2], bf, addr_space="Shared")
          for l in range(3)]

    # segment-tail schedule (same for both roles by construction)
    seg_bounds = []
    b0 = 0
    for sz in ([4] * 3 + [8] * 2 + [16] * 2):
        if b0 >= NLOC:
            break
        seg_bounds.append((b0, min(b0 + sz, NLOC)))
        b0 += sz
    assert seg_bounds[-1][1] == NLOC
    seg_last = {}
    for si, (sb0, sb1) in enumerate(seg_bounds):
        last = 0
        for ci, adds in enumerate(chunk_adds):
            if any(sb < sb1 and sb + nb > sb0 for (_, sb, nb) in adds):
                last = max(last, ci)
        seg_last[si] = last

    with tile.TileContext(nc, num_cores=tile_cores) as tc:
        with (
            tc.tile_pool(name="const", bufs=1) as cpool,
            tc.tile_pool(name="state", bufs=1) as spool,
            tc.tile_pool(name="xw", bufs=xw_bufs) as xpool,
            tc.tile_pool(name="idx", bufs=idx_bufs) as ipool,
            tc.tile_pool(name="gath", bufs=gb_bufs) as gpool,
            tc.tile_pool(name="w1t", bufs=w1_bufs) as wpool,
            tc.tile_pool(name="dram", bufs=1, space="DRAM") as dpool,
            tc.tile_pool(name="psum_t", bufs=pt_bufs, space="PSUM") as pt_pool,
            tc.tile_pool(name="psum_tr", bufs=ptr_bufs, space="PSUM") as ptr_pool,
            tc.tile_pool(name="psum_g", bufs=1, space="PSUM") as pg_pool,
            tc.tile_pool(name="psum_z", bufs=1, space="PSUM") as pz_pool,
        ):
            dinv64 = cpool.tile([P, NLOC, HID], f32, tag="dinv64")
            nc.sync.dma_start(out=dinv64[:], in_=dinv_d[:].rearrange(
                "p (g f) -> p g f", f=HID))
            mask2 = cpool.tile([P, NLOC, 2], f32, tag="mask2")
            nc.sync.dma_start(out=mask2[:], in_=mask_d[:].rearrange(
                "p (g t) -> p g t", t=2))
            ident = cpool.tile([P, P], f32, tag="ident")
            nc.sync.dma_start(out=ident[:], in_=ident_d[:])
            Wc_sb, bc_sb, wl_sb = [], [], []
            for i in range(3):
                w = cpool.tile([FIN if i == 0 else HID, HID], bf, tag=f"Wc{i}")
                nc.sync.dma_start(out=w[:], in_=Wc_d[i][:])
                Wc_sb.append(w)
                b = cpool.tile([HID, 1], f32, tag=f"bc{i}")
                nc.sync.dma_start(out=b[:], in_=bc_d[i][:])
                bc_sb.append(b)
                wl = cpool.tile([HID, 1], bf, tag=f"wl{i}")
                nc.sync.dma_start(out=wl[:], in_=wl_d[i][:])
                wl_sb.append(wl)
            sidx_t = cpool.tile([P, NLOC * P // 16], mybir.dt.int16,
                                tag="sidx")
            nc.sync.dma_start(out=sidx_t[:], in_=sidx_d[:])
            gidx_t = cpool.tile([P, Eprime // 16], mybir.dt.int16,
                                tag="gidx")
            nc.sync.dma_start(out=gidx_t[:], in_=gidx_d[:])

            # ---- zero the shared tensors, then the startup barrier
            ZG = 8
            zero_t = cpool.tile([P, ZG, F2], bf, tag="zero")
            nc.vector.memset(zero_t[:], 0.0)
            zero_dmas = []
            for l in range(3):
                for off in range(0, NSLOT2, ZG):
                    zi = nc.sync.dma_start(
                        out=sh[l][:].rearrange("(g p) f -> p g f",
                                               p=P)[:, off:off + ZG, :],
                        in_=zero_t[:])
                    zero_dmas.append(zi)
            tiny = cpool.tile([1, 4], f32, tag="tiny")
            nc.vector.memset(tiny[:], 1.0)
            bar_state = dict(k=0)

            def barrier(dep_insts, name):
                k = bar_state["k"]
                bar_state["k"] = k + 1
                bar_i = dpool.tile([1, 4], f32, tag=f"bar_i{k}")
                bar_o = dpool.tile([2, 4], f32, tag=f"bar_o{k}")
                nc.gpsimd.dma_start(bar_i[:], tiny[:])
                cc = nc.gpsimd.collective_compute(
                    "AllGather", mybir.AluOpType.bypass, PAIRS,
                    ins=[bar_i.opt()], outs=[bar_o.opt()])
                for d in dep_insts:
                    add_dep_helper(cc.ins, d.ins, sync=True,
                                   reason=f"{name}<-dep")
                return cc

            cc0 = barrier(zero_dmas, "cc0")

            m2_sb = spool.tile([P, NLOC, F2], bf, tag="m2")
            s2_sb = spool.tile([P, NLOC, F2], f32, tag="s2")
            h2 = spool.tile([HID, NLOC * 2 * P], bf, tag="h2")
            g_acc = spool.tile([P, NLOC, 2], bf, tag="g_acc")
            nc.vector.memset(g_acc[:], 0.0)

            psum_z = pz_pool.tile([2, NFC], f32, tag="pz")
            head_state = dict(emitted=0, total=NLOC)
            scatters = {0: [], 1: [], 2: []}
            psum_g_ref = [None]

            def emit_gemm_group(l, g0, gn):
                psum_t = pt_pool.tile([P, gemm_grp, F2], f32, tag="pt")
                if l == 0:
                    xt = xpool.tile([FIN, 2, gemm_grp, P], bf, tag="xt")
                    nc.sync.dma_start(
                        out=xt[:, :, :gn, :],
                        in_=x2_d[:].rearrange("f (s g p) -> f s g p",
                                              s=2, p=P)[:, :, g0:g0 + gn, :])
                for j in range(gn):
                    for s in range(2):
                        if l == 0:
                            lhsT = xt[:, s, j, :]
                        else:
                            lhsT = h2[:, ((g0 + j) * 2 + s) * P:
                                      ((g0 + j) * 2 + s + 1) * P]
                        nc.tensor.matmul(
                            psum_t[:, j, s * HID:(s + 1) * HID],
                            lhsT, Wc_sb[l][:], start=True, stop=True)
                for s in range(2):
                    nc.vector.tensor_mul(
                        m2_sb[:, g0:g0 + gn, s * HID:(s + 1) * HID],
                        psum_t[:, :gn, s * HID:(s + 1) * HID],
                        dinv64[:, g0:g0 + gn, :])
                sc = nc.gpsimd.dma_scatter_add(
                    sh[l][:], m2_sb[:, g0:g0 + gn, :],
                    sidx_t[:, g0 * 8:(g0 + gn) * 8],
                    gn * P, gn * P, F2, single_packet=False)
                add_dep_helper(sc.ins, cc0.ins, sync=True, reason="cc0->sc")
                scatters[l].append(sc)
                nc.vector.tensor_copy(s2_sb[:, g0:g0 + gn, :],
                                      m2_sb[:, g0:g0 + gn, :])

            def emit_head_seg(b0, b1):
                psum_g = psum_g_ref[0]
                nc.vector.tensor_add(g_acc[:, b0:b1, :], g_acc[:, b0:b1, :],
                                     psum_g[:, b0:b1, :])
                nc.vector.tensor_mul(g_acc[:, b0:b1, :], g_acc[:, b0:b1, :],
                                     mask2[:, b0:b1, :])
                for g0 in range(b0, b1, hgrp):
                    gn = min(hgrp, b1 - g0)
                    w1t = wpool.tile([P, hgrp, NFC], bf, tag="w1t")
                    nc.sync.dma_start(
                        out=w1t[:, :gn, :],
                        in_=w1t_d[:].rearrange(
                            "(g p) f -> p g f", p=P)[:, g0:g0 + gn, :])
                    for j in range(gn):
                        nc.tensor.matmul(
                            psum_z[:], g_acc[:, g0 + j, :], w1t[:, j, :],
                            start=(head_state["emitted"] == 0),
                            stop=(head_state["emitted"] ==
                                  head_state["total"] - 1))
                        head_state["emitted"] += 1

            def emit_seg_tail(l, b0, b1):
                for s in range(2):
                    nc.vector.tensor_mul(
                        s2_sb[:, b0:b1, s * HID:(s + 1) * HID],
                        s2_sb[:, b0:b1, s * HID:(s + 1) * HID],
                        dinv64[:, b0:b1, :])
                for g0 in range(b0, b1, tr_grp):
                    gn = min(tr_grp, b1 - g0)
                    psum_tr = ptr_pool.tile([HID, tr_grp, 2, P], f32,
                                            tag="ptr")
                    for j in range(gn):
                        for s in range(2):
                            nc.tensor.transpose(
                                psum_tr[:, j, s, :],
                                s2_sb[:, g0 + j, s * HID:(s + 1) * HID],
                                ident[:])
                    nc.scalar.activation(
                        h2[:, g0 * 2 * P:(g0 + gn) * 2 * P],
                        psum_tr[:, :gn, :, :].rearrange(
                            "f g s p -> f (g s p)"),
                        mybir.ActivationFunctionType.Relu,
                        bias=bc_sb[l][:], scale=1.0)
                psum_g = psum_g_ref[0]
                for j in range(b0, b1):
                    for s in range(2):
                        nc.tensor.matmul(
                            psum_g[:, j, s:s + 1],
                            h2[:, (j * 2 + s) * P:(j * 2 + s + 1) * P],
                            wl_sb[l][:], start=True, stop=True)
                if not PIPELINE:
                    return
                if l < 2:
                    for g0 in range(b0, b1, gemm_grp):
                        emit_gemm_group(l + 1, g0, min(gemm_grp, b1 - g0))
                elif TAIL_LEVEL >= 3:
                    emit_head_seg(b0, b1)

            for l in range(3):
                if l == 0 or not PIPELINE:
                    for g0 in range(0, NLOC, gemm_grp):
                        emit_gemm_group(l, g0, min(gemm_grp, NLOC - g0))
                cc_l = barrier(scatters[l], f"cc{l + 1}")
                psum_g = pg_pool.tile([P, NLOC, 2], f32, tag="pg")
                psum_g_ref[0] = psum_g
                for ci, (cstart, clen) in enumerate(chunks):
                    cblk = clen // P
                    gbuf = gpool.tile([P, (CHUNK + P - 1) // P, F2], bf,
                                      tag="gb")
                    gi = nc.gpsimd.dma_gather(
                        gbuf[:, :cblk, :], sh[l][:],
                        gidx_t[:, cstart // 16:(cstart + clen) // 16],
                        clen, clen, F2, single_packet=False)
                    add_dep_helper(gi.ins, cc_l.ins, sync=True,
                                   reason="cc->gather")
                    for (gb, sb, nb) in chunk_adds[ci]:
                        nc.vector.tensor_add(s2_sb[:, sb:sb + nb, :],
                                             s2_sb[:, sb:sb + nb, :],
                                             gbuf[:, gb:gb + nb, :])
                    if PIPELINE:
                        for si, (sb0, sb1) in enumerate(seg_bounds):
                            if seg_last[si] == ci:
                                emit_seg_tail(l, sb0, sb1)
                if not PIPELINE:
                    emit_seg_tail(l, 0, NLOC)
                    if l == 2:
                        emit_head_seg(0, NLOC)
                if l < 2:
                    nc.vector.tensor_add(g_acc[:], g_acc[:], psum_g_ref[0][:])

            assert head_state["emitted"] == NLOC, head_state
            z_sb = spool.tile([2, NFC], f32, tag="z")
            nc.vector.tensor_copy(z_sb[:], psum_z[:])
            nc.sync.dma_start(out=z_d[:], in_=z_sb[:])

    nc.compile()
    return nc


def make_in_maps(prep, shared, per_core):
    eye = np.eye(P, dtype=np.float32)
    maps = []
    for c in range(N_CORES):
        pc = per_core[c]
        m = dict(
            x2=np.ascontiguousarray(pc["x2"].reshape(FIN, -1)),
            dinv64=pc["dinv64"], mask2=pc["mask2"],
            gidx=pc["gidx"], sidx=pc["sidx"], W1T=pc["W1T"], ident=eye,
        )
        for i in range(3):
            m[f"Wc{i}"] = shared["Wc"][i]
            m[f"bc{i}"] = shared["bc2"][i]
            m[f"wl{i}"] = shared["wl2"][i]
        maps.append(m)
    return maps


def finish_host(z_all, inputs):
    fc_b = float(np.asarray(inputs["fc_b"], np.float32).reshape(()))
    lin1_W = np.asarray(inputs["lin1_W"], np.float32)
    b1_eff = (np.asarray(inputs["lin1_b"], np.float32)
              + fc_b * lin1_W.sum(axis=1))
    W2 = np.asarray(inputs["lin2_W"], np.float32)
    b2 = np.asarray(inputs["lin2_b"], np.float32)
    bs = len(z_all)
    z = np.zeros((bs, NFC), np.float32)
    for k in range(bs // 2):
        zsum = z_all[2 * k] + z_all[2 * k + 1]
        z[2 * k] = zsum[0]
        z[2 * k + 1] = zsum[1]
    z = np.maximum(z + b1_eff, 0.0)
    logits = z @ W2.T + b2
    mx = logits.max(axis=1, keepdims=True)
    e = np.exp(logits - mx)
    return ((logits - mx) - np.log(e.sum(axis=1, keepdims=True))).astype(
        np.float32)


_PROGRAM_CACHE = {}


def _get_program(prep, cache_key):
    hit = _PROGRAM_CACHE.get(cache_key)
    if hit is None:
        hit = build_program(prep)
        _PROGRAM_CACHE[cache_key] = hit
    return hit


def kernel(**inputs) -> np.ndarray:
    x = np.asarray(inputs["x"])
    bs, n = x.shape[0], x.shape[1]
    assert bs == N_CORES, f"expected batch {N_CORES}, got {bs}"

    edge_index = np.asarray(inputs["edge_index"])
    prep = preprocess(n, edge_index)
    cache_key = (n, edge_index.shape[1], hash(edge_index.tobytes()))
    nc = _get_program(prep, cache_key)
    shared, per_core = build_constants(prep, inputs)
    in_maps = make_in_maps(prep, shared, per_core)

    last_err = None
    for attempt in range(3):
        try:
            res = run_bass_kernel_spmd(nc, in_maps, list(range(N_CORES)))
            break
        except Exception as e:
            last_err = e
    else:
        raise last_err
    z_all = [res.results[c]["z"] for c in range(N_CORES)]
    return finish_host(z_all, inputs)
